# revision 20
# baseline (speedup 1.0000x reference)
"""Trainium2 Bass kernel for nn_AttributeBiasLoss.

Reference computation:
    per_node = mean(sigmoid(predictions), axis=1)            # [B]
    for each attribute a: group per_node by attr_vals[:, a] (V=16 values)
    means[a, v] = mean of per_node over group (a, v)
    loss = sum over attrs of pairwise squared diffs of present group means
           / number of comparisons

Kernel strategy (data-parallel over 8 cores, batch-sharded):
  Host casts preds to fp16 and attrs to fp16 pre-scaled by 8 (halves DMA).
  Device packs z = 8*attr + ys (ys = sum of 8 sigmoids, in (0,8)) so every
  knot k_v = 8(v+1) separates buckets exactly.  Knot sums M_v = sum min(z,k)
  and prefix counts P_v are extracted with one accumulating pass per (a,v):
    - DVE: custom dual-stream ops (min2 / lt2), ~1.18us per knot
    - ACT: reflected relu  relu(k - z)  (M_v = k*n - sum relu) and Sign
      counts, ~2.2us per knot
  min- and relu-family knots interconvert per-partition with compile-time
  constants, so knots are assigned to engines at single-knot granularity for
  balance.  The Pool engine builds z via tensor_tensor adds.  Knot columns
  use a gapped [A, 17] layout so all adjacent-knot differences collapse into
  one wide shifted-subtract plus a sign/const fix (per-instruction overhead
  dominates narrow ops).  Cross-partition reduce via PE ones-matmul,
  AllReduce of the tiny [1,256] stats, redundant per-core epilogue (centered
  variance form) producing the scalar loss.

  NOTE: the Tile framework records cross-engine RAW deps at emission time —
  every producer (e.g. the Pool z-build) must be emitted before any consumer
  on another engine, or the consumer races ahead and reads stale zeros.
"""

import sys

sys.path.insert(0, "/opt/trn_rl_repo")

from contextlib import ExitStack

import numpy as np

import concourse.bacc as bacc
import concourse.bass as bass
import concourse.mybir as mybir
import concourse.tile as tile
from concourse import bass_utils
from concourse._compat import with_exitstack

# ---------------------------------------------------------------------------
# Custom two-stream knot-reduce DVE ops (dual rd ports -> 2 elems/cycle).
# MIN2_KNOT_ANT:  out = min(src0, c0) + min(src1, c0); accum_out = c1 + sum(out)
# LT2_KNOT_ANT:   out = (src0 < c0) + (src1 < c0);     accum_out = c1 + sum(out)
# ---------------------------------------------------------------------------
import concourse.dve_ops as dve_ops
from concourse.dve_ops import DveOp
from concourse.dve_spec import C0, C1, Spec, Src0, Src1, lower, minn, _has_src1
from concourse.dve_uop import DveOpSpec


def _ref_body_sum_c1(body_fn):
    def _r(in0, in1, c0, c1, c2):
        b = body_fn(in0, in1, c0, c1, c2).astype(np.float32)
        return b, c1 + b.reshape(b.shape[0], -1).sum(axis=-1, keepdims=True)

    return _r


def _make_op(name: str, spec: Spec) -> DveOp:
    row = max(dve_ops._SUB_OPCODE_FOR_NAME.values()) + 1
    assert row < 0x20
    dve_ops._SUB_OPCODE_FOR_NAME[name] = row
    shas = {}
    for ver in ("v3", "v4"):
        uops = lower(spec, ver=ver)
        shas[ver] = DveOpSpec(
            name=name, opcode=row, uops=uops, rd1_en=_has_src1(spec)
        ).sha(ver)
    op = DveOp(name, spec, subdim=False, uops_sha=shas)
    dve_ops.OPS.append(op)
    dve_ops.CUSTOM_DVE_SPECS[name] = spec
    return op


def _ref_min2(in0, in1, c0, c1, c2):
    return np.minimum(in0.astype(np.float32), c0) + np.minimum(
        in1.astype(np.float32), c0
    )


def _ref_lt2(in0, in1, c0, c1, c2):
    return (in0.astype(np.float32) < c0).astype(np.float32) + (
        in1.astype(np.float32) < c0
    ).astype(np.float32)


_registered = {}


def get_ops():
    if not _registered:
        _registered["min2"] = _make_op(
            "MIN2_KNOT_ANT",
            Spec(
                body=minn(Src0, C0) + minn(Src1, C0),
                accum=lambda a, b: a + b,
                accum_init=C1,
                reference=_ref_body_sum_c1(_ref_min2),
            ),
        )
        _registered["lt2"] = _make_op(
            "LT2_KNOT_ANT",
            Spec(
                body=(Src0 < C0) + (Src1 < C0),
                accum=lambda a, b: a + b,
                accum_init=C1,
                reference=_ref_body_sum_c1(_ref_lt2),
            ),
        )
    return _registered


F32 = mybir.dt.float32
F16 = mybir.dt.float16
AF = mybir.ActivationFunctionType
OP = mybir.AluOpType

# Problem constants (hardcoded per harness contract).
B, D, A, V = 2_000_000, 8, 8, 16
NCORES = 8
ROWS_PER_CORE = B // NCORES  # 250_000

SUB = 10
CP = 1960  # per-partition columns; 128*CP = 250_880 rows padded
CSUB = CP // SUB
ROWS_PAD = 128 * CP

# Engine assignment (balance knobs).
# Sums: per attr, knots v < K_DVE[a] on DVE min2 (as M_v), the rest on ACT
# reflected relu (as R_v = sum relu(k_v - z)).
K_SPLIT = 5  # per attr: knots v < K_SPLIT on DVE min2, rest on ACT relu
# Counts: attr ACT_CNT_A, v < ACT_CNT_N on ACT Sign; the rest on DVE lt2.
ACT_CNT_A = 7
ACT_CNT_N = 8

W = V + 1  # sum block stride per attr (gap col at w=0 for coalesced diffs)
NSUM = A * W  # 136 sum cols incl gaps
NCNT = A * (V - 1)  # 120 count cols
NACC = NSUM + NCNT  # 256


@with_exitstack
def emit_kernel(
    ctx: ExitStack,
    tc: tile.TileContext,
    pred_d,  # DRAM [ROWS_PAD, D] f16
    attr_d,  # DRAM [A, ROWS_PAD] f16 (attribute-major, host 8x-scaled)
    loss_d,  # DRAM [1, 1] f32
    dbg_d=None,  # DRAM [1, NACC] f32 (post-AllReduce stats, debug)
):
    nc = tc.nc
    cp, sub, csub = CP, SUB, CSUB
    n_real = float(ROWS_PER_CORE * NCORES)
    npad_tot = float((ROWS_PAD - ROWS_PER_CORE) * NCORES)
    n_slots = float(ROWS_PAD * NCORES)  # 1960*128*8 incl pads

    io = ctx.enter_context(tc.tile_pool(name="io", bufs=3))
    predp = ctx.enter_context(tc.tile_pool(name="pred", bufs=sub))
    zp = ctx.enter_context(tc.tile_pool(name="z", bufs=1))
    accp = ctx.enter_context(tc.tile_pool(name="acc", bufs=1))
    junkp = ctx.enter_context(tc.tile_pool(name="junk", bufs=1))
    smallp = ctx.enter_context(tc.tile_pool(name="small", bufs=1))
    psump = ctx.enter_context(tc.tile_pool(name="ps", bufs=1, space="PSUM"))
    dramp = ctx.enter_context(tc.tile_pool(name="dram", bufs=1, space="DRAM"))

    x16 = zp.tile([128, A * cp], F16, name="x16")  # 8*attr, fp16
    x16_a = x16.rearrange("p (a c) -> p a c", a=A)
    z32 = zp.tile([128, A * cp], F32, name="z32")  # 8*attr + ys
    z32_a = z32.rearrange("p (a c) -> p a c", a=A)
    ys = zp.tile([128, cp], F32, name="ys")  # sum of 8 sigmoids, (0,8)

    acc = accp.tile([128, NACC], F32, name="acc")
    junk_h = junkp.tile([128, cp // 2], F32, name="junk_h")
    junk_a = junkp.tile([128, cp], F32, name="junk_a")
    junk_s = junkp.tile([128, cp], F16, name="junk_s")

    # ACT bias columns: relu knots +8(v+1); sign thresholds -(8v+4)
    kbias = smallp.tile([128, V], F32, name="kbias")
    for v in range(V):
        nc.vector.memset(kbias[:, v : v + 1], float(8 * (v + 1)))
    sbias = smallp.tile([128, V - 1], F32, name="sbias")
    for v in range(V - 1):
        nc.vector.memset(sbias[:, v : v + 1], -float(8 * v + 4))

    # sign/const vectors for the coalesced diff fix, periodic per attr block
    kk_ = K_SPLIT
    sgn_t = smallp.tile([128, NSUM], F32, name="sgn_t")
    cvec_t = smallp.tile([128, NSUM], F32, name="cvec_t")
    sgn_av = sgn_t.rearrange("p (a w) -> p a w", a=A)
    cvec_av = cvec_t.rearrange("p (a w) -> p a w", a=A)
    nc.vector.memset(sgn_av[:, :, 0:1], 0.0)
    nc.vector.memset(sgn_av[:, :, 1 : 1 + kk_], 1.0)
    nc.vector.memset(sgn_av[:, :, 1 + kk_ : W], -1.0)
    nc.vector.memset(cvec_av[:, :, 0 : 1 + kk_], 0.0)
    nc.vector.memset(cvec_av[:, :, 1 + kk_ : 2 + kk_], float(8 * (kk_ + 1) * CP))
    nc.vector.memset(cvec_av[:, :, 2 + kk_ : W], float(8 * CP))
    # zero the gap columns of acc (read by the shifted diff)
    acc_gap = acc[:, 0:NSUM].rearrange("p (a w) -> p a w", a=A)
    nc.vector.memset(acc_gap[:, :, 0:1], 0.0)

    # Warm up the collective channel early (overlaps with main compute).
    warm_in = dramp.tile([1, NACC], F32, name="warm_in")
    warm_out = dramp.tile([1, NACC], F32, name="warm_out")
    warm_s = smallp.tile([1, NACC], F32, name="warm_s")
    nc.vector.memset(warm_s[:], 0.0)
    nc.sync.dma_start(warm_in[:], warm_s[:])
    nc.gpsimd.collective_compute(
        "AllReduce",
        OP.add,
        replica_groups=[list(range(NCORES))],
        ins=[warm_in.opt()],
        outs=[warm_out.opt()],
    )

    # ---- input DMA: interleave attr columns with pred chunks so both the
    # count stream (DVE, needs attr_0 first) and the sigmoid pipeline (ACT,
    # needs pred chunks) start immediately.
    attr_v = attr_d.rearrange("a (p c) -> a p c", p=128)
    pred_v = pred_d.rearrange("(p s c) d -> s p (c d)", p=128, s=sub)
    pred_ts = [
        predp.tile([128, csub * D], F16, tag="pred", name=f"pred{s}")
        for s in range(sub)
    ]
    nc.sync.dma_start(x16_a[:, 0, :], attr_v[0])
    nc.sync.dma_start(pred_ts[0][:], pred_v[0])
    nc.sync.dma_start(pred_ts[1][:], pred_v[1])
    ai, si = 1, 2
    while ai < A or si < sub:
        if ai < A:
            nc.sync.dma_start(x16_a[:, ai, :], attr_v[ai])
            ai += 1
        if si < sub:
            nc.sync.dma_start(pred_ts[si][:], pred_v[si])
            si += 1

    # ---- emission in topological order: the Tile framework records RAW
    # deps at emission time, so every producer must be emitted before its
    # consumers (per-engine queues keep their own program order).

    # ACT: sigmoid chunks -> sig32
    sig_ts = []
    for s in range(sub):
        st = io.tile([128, csub * D], F32, tag="sig")
        nc.scalar.activation(st[:], pred_ts[s][:], AF.Sigmoid)
        sig_ts.append(st)

    cops = get_ops()
    H = cp // 2
    k = K_SPLIT

    def sum_col(a, v):
        return a * W + 1 + v  # gap col at a*W

    def cnt_col(a, v):
        return NSUM + a * (V - 1) + v

    cnt_jobs = []  # (a, v) on DVE lt2
    for a in range(A):
        for v in range(V - 1):
            if a == ACT_CNT_A and v < ACT_CNT_N:
                continue
            cnt_jobs.append((a, v))

    def emit_cnt(job):
        a, v = job
        xa = x16_a[:, a, :]
        nc.vector._custom_dve(
            cops["lt2"],
            out=junk_h[:],
            in0=xa[:, 0:H],
            in1=xa[:, H:cp],
            s0=float(8 * v + 4),
            s1=0.0,
            accum_out=acc[:, cnt_col(a, v) : cnt_col(a, v) + 1],
        )

    # DVE: interleave early counts with the y-reduce chunks.
    ji = 0
    for s in range(sub):
        if ji < len(cnt_jobs):
            emit_cnt(cnt_jobs[ji])
            ji += 1
        t1 = ys[:, s * csub : (s + 1) * csub]
        nc.vector.tensor_reduce(
            t1,
            sig_ts[s].rearrange("p (c d) -> p c d", d=D),
            op=OP.add,
            axis=mybir.AxisListType.X,
        )

    # z_a = x16_a + ys (fp16 + fp32 -> fp32) on the Pool engine; emitted
    # before any z consumer.
    Z_ON_POOL = True
    zeng = nc.gpsimd if Z_ON_POOL else nc.vector
    for a in range(A):
        zeng.tensor_tensor(
            out=z32_a[:, a, :], in0=x16_a[:, a, :], in1=ys[:], op=OP.add
        )

    # ACT: Sign count knots (fill ACT idle while z is being built)...
    for v in range(ACT_CNT_N):
        nc.scalar.activation(
            junk_s[:],
            x16_a[:, ACT_CNT_A, :],
            AF.Sign,
            bias=sbias[:, v : v + 1],
            accum_out=acc[:, cnt_col(ACT_CNT_A, v) : cnt_col(ACT_CNT_A, v) + 1],
        )
    # ...then ACT sum knots: reflected relu R_v = sum relu(k_v - z), attr-major.
    for a in range(A):
        for v in range(k, V):
            nc.scalar.activation(
                junk_a[:],
                z32_a[:, a, :],
                AF.Relu,
                bias=kbias[:, v : v + 1],
                scale=-1.0,
                accum_out=acc[:, sum_col(a, v) : sum_col(a, v) + 1],
            )

    # DVE: remaining counts, then min2 sum knots (v < k), attr-major.
    while ji < len(cnt_jobs):
        emit_cnt(cnt_jobs[ji])
        ji += 1
    for a in range(A):
        za = z32_a[:, a, :]
        for v in range(k):
            nc.vector._custom_dve(
                cops["min2"],
                out=junk_h[:],
                in0=za[:, 0:H],
                in1=za[:, H:cp],
                s0=float(8 * (v + 1)),
                s1=0.0,
                accum_out=acc[:, sum_col(a, v) : sum_col(a, v) + 1],
            )

    # ---- coalesced diffs over the gapped sum block -----------------------
    # Shifted subtract d_w = acc_w - acc_{w-1} gives, per attr (gap=0 col):
    #   v < k:  M_v - M_{v-1}            -> true dacc        (sgn +1, c 0)
    #   v = k:  R_k - M_{k-1}            -> need 8(k+1)cp - R_k - M_{k-1}
    #           = -(d) + 8(k+1)cp - 2 M_{k-1}   (sgn -1, c 8(k+1)cp, fixup)
    #   v > k:  R_v - R_{v-1}            -> -(d) + 8cp       (sgn -1, c 8cp)
    dacc = accp.tile([128, NACC], F32, name="dacc")
    nc.vector.memset(dacc[:, 0:1], 0.0)
    nc.vector.tensor_tensor(
        out=dacc[:, 1:NSUM], in0=acc[:, 1:NSUM], in1=acc[:, 0 : NSUM - 1],
        op=OP.subtract,
    )
    nc.vector.tensor_tensor(
        out=dacc[:, 0:NSUM], in0=dacc[:, 0:NSUM], in1=sgn_t[:], op=OP.mult
    )
    nc.vector.tensor_tensor(
        out=dacc[:, 0:NSUM], in0=dacc[:, 0:NSUM], in1=cvec_t[:], op=OP.add
    )
    # boundary fixup: dacc[bcol] -= 2*M_{k-1} at bcol = a*W+1+k (stride W)
    dacc_av = dacc[:, 0:NSUM].rearrange("p (a w) -> p a w", a=A)
    acc_av = acc[:, 0:NSUM].rearrange("p (a w) -> p a w", a=A)
    nc.vector.scalar_tensor_tensor(
        out=dacc_av[:, :, 1 + k : 2 + k],
        in0=acc_av[:, :, k : k + 1],
        scalar=-2.0,
        in1=dacc_av[:, :, 1 + k : 2 + k],
        op0=OP.mult,
        op1=OP.add,
    )
    nc.vector.tensor_copy(dacc[:, NSUM:NACC], acc[:, NSUM:NACC])

    # ---- cross-partition reduce + AllReduce ------------------------------
    ones_t = smallp.tile([128, 1], F32, name="ones")
    nc.vector.memset(ones_t[:], 1.0)
    red_ps = psump.tile([1, NACC], F32)
    nc.tensor.matmul(red_ps[:], lhsT=ones_t[:], rhs=dacc[:], start=True, stop=True)
    core_stats = smallp.tile([1, NACC], F32, name="core_stats")
    nc.vector.tensor_copy(core_stats[:], red_ps[:])

    cc_in = dramp.tile([1, NACC], F32, name="cc_in")
    cc_out = dramp.tile([1, NACC], F32, name="cc_out")
    nc.sync.dma_start(cc_in[:], core_stats[:])
    nc.gpsimd.collective_compute(
        "AllReduce",
        OP.add,
        replica_groups=[list(range(NCORES))],
        ins=[cc_in.opt()],
        outs=[cc_out.opt()],
    )
    g = smallp.tile([1, NACC], F32, name="g")
    nc.sync.dma_start(g[:], cc_out[:])
    if dbg_d is not None:
        nc.sync.dma_start(dbg_d[:], g[:])

    # ---------------- epilogue (tiny, partition 0, redundant per core) -----
    ep = ctx.enter_context(tc.tile_pool(name="ep", bufs=1))
    gS = g[:, 0:NSUM].rearrange("p (a w) -> p a w", a=A)[
        :, :, 1:W
    ]  # dM_glob = 8(S+Cge+pad), gap col dropped
    gC = g[:, NSUM:NACC].rearrange("p (a v) -> p a v", a=A)  # count raw

    # P[a, v] for v<15 (prefix counts, pad-free); ACT sign cols converted.
    P = ep.tile([1, A * (V - 1)], F32, name="P").rearrange("p (a v) -> p a v", a=A)
    nc.vector.tensor_copy(P[:], gC[:])
    nc.vector.tensor_scalar(
        out=P[:, ACT_CNT_A, 0:ACT_CNT_N],
        in0=gC[:, ACT_CNT_A, 0:ACT_CNT_N],
        scalar1=-0.5,
        scalar2=n_slots / 2.0,
        op0=OP.mult,
        op1=OP.add,
    )

    # n_v: n_0 = P_0; n_v = P_v - P_{v-1}; n_15 = n_real - P_14
    n_t = ep.tile([1, A * V], F32, name="n").rearrange("p (a v) -> p a v", a=A)
    nc.vector.tensor_copy(n_t[:, :, 0:1], P[:, :, 0:1])
    nc.vector.tensor_tensor(
        out=n_t[:, :, 1 : V - 1], in0=P[:, :, 1 : V - 1], in1=P[:, :, 0 : V - 2],
        op=OP.subtract,
    )
    nc.vector.tensor_scalar(
        out=n_t[:, :, V - 1 : V], in0=P[:, :, V - 2 : V - 1],
        scalar1=-1.0, scalar2=n_real, op0=OP.mult, op1=OP.add,
    )

    # S_v = gS/8 - (n_real - P_v) - npad_tot   (P_15 := n_real)
    S = ep.tile([1, A * V], F32, name="S").rearrange("p (a v) -> p a v", a=A)
    nc.vector.tensor_scalar(
        out=S, in0=gS, scalar1=0.125, scalar2=-(n_real + npad_tot),
        op0=OP.mult, op1=OP.add,
    )
    nc.vector.tensor_tensor(
        out=S[:, :, 0 : V - 1], in0=S[:, :, 0 : V - 1], in1=P[:], op=OP.add
    )
    nc.vector.tensor_scalar(
        out=S[:, :, V - 1 : V], in0=S[:, :, V - 1 : V],
        scalar1=n_real, scalar2=None, op0=OP.add,
    )

    # m = S / max(n, 1)
    nmax = ep.tile([1, A * V], F32, name="nmax")
    nc.vector.tensor_scalar(
        out=nmax[:], in0=n_t.rearrange("p a v -> p (a v)"),
        scalar1=1.0, scalar2=None, op0=OP.max,
    )
    rn = ep.tile([1, A * V], F32, name="rn")
    nc.vector.reciprocal(rn[:], nmax[:])
    m = ep.tile([1, A * V], F32, name="m").rearrange("p (a v) -> p a v", a=A)
    nc.vector.tensor_tensor(
        out=m, in0=S, in1=rn.rearrange("p (a v) -> p a v", a=A), op=OP.mult
    )

    # present mask & per-attr stats
    p_t = ep.tile([1, A * V], F32, name="p").rearrange("p (a v) -> p a v", a=A)
    nc.vector.tensor_scalar(out=p_t, in0=n_t, scalar1=0.5, scalar2=None, op0=OP.is_ge)
    k_t = ep.tile([1, A], F32, name="k")
    nc.vector.tensor_reduce(k_t[:], p_t, op=OP.add, axis=mybir.AxisListType.X)

    mp_t = ep.tile([1, A * V], F32, name="mp").rearrange("p (a v) -> p a v", a=A)
    nc.vector.tensor_tensor(out=mp_t, in0=m, in1=p_t, op=OP.mult)
    ms = ep.tile([1, A], F32, name="ms")
    nc.vector.tensor_reduce(ms[:], mp_t, op=OP.add, axis=mybir.AxisListType.X)

    kmax = ep.tile([1, A], F32, name="kmax")
    nc.vector.tensor_scalar(
        out=kmax[:], in0=k_t[:], scalar1=1.0, scalar2=None, op0=OP.max
    )
    rk = ep.tile([1, A], F32, name="rk")
    nc.vector.reciprocal(rk[:], kmax[:])
    mu = ep.tile([1, A], F32, name="mu")
    nc.vector.tensor_tensor(out=mu[:], in0=ms[:], in1=rk[:], op=OP.mult)

    # d = (m - mu) * present ; q = sum_v d^2 ; contrib = k * q
    dtile = ep.tile([1, A * V], F32, name="d").rearrange("p (a v) -> p a v", a=A)
    nc.vector.scalar_tensor_tensor(
        out=dtile, in0=mu[:].broadcast_to([1, A, V]), scalar=-1.0, in1=m,
        op0=OP.mult, op1=OP.add,
    )
    nc.vector.tensor_tensor(out=dtile, in0=dtile, in1=p_t, op=OP.mult)
    d2 = ep.tile([1, A * V], F32, name="d2").rearrange("p (a v) -> p a v", a=A)
    nc.vector.tensor_tensor(out=d2, in0=dtile, in1=dtile, op=OP.mult)
    q_t = ep.tile([1, A], F32, name="q")
    nc.vector.tensor_reduce(q_t[:], d2, op=OP.add, axis=mybir.AxisListType.X)

    contrib = ep.tile([1, A], F32, name="contrib")
    nc.vector.tensor_tensor(out=contrib[:], in0=k_t[:], in1=q_t[:], op=OP.mult)
    tot = ep.tile([1, 1], F32, name="tot")
    nc.vector.tensor_reduce(tot[:], contrib[:], op=OP.add, axis=mybir.AxisListType.X)

    # ncomp = sum_a k(k-1)/2
    kk = ep.tile([1, A], F32, name="kk")
    nc.vector.scalar_tensor_tensor(
        out=kk[:], in0=k_t[:], scalar=-1.0, in1=k_t[:], op0=OP.add, op1=OP.mult
    )
    ncomp = ep.tile([1, 1], F32, name="ncomp")
    nc.vector.tensor_reduce(ncomp[:], kk[:], op=OP.add, axis=mybir.AxisListType.X)
    nc.vector.tensor_scalar(
        out=ncomp[:], in0=ncomp[:], scalar1=0.5, scalar2=None, op0=OP.mult
    )

    # loss = (ncomp > 0) * tot / max(ncomp, 0.5)
    ncm = ep.tile([1, 1], F32, name="ncm")
    nc.vector.tensor_scalar(
        out=ncm[:], in0=ncomp[:], scalar1=0.5, scalar2=None, op0=OP.max
    )
    rnc = ep.tile([1, 1], F32, name="rnc")
    nc.vector.reciprocal(rnc[:], ncm[:])
    mask = ep.tile([1, 1], F32, name="mask")
    nc.vector.tensor_scalar(
        out=mask[:], in0=ncomp[:], scalar1=0.25, scalar2=None, op0=OP.is_ge
    )
    res = ep.tile([1, 1], F32, name="res")
    nc.vector.tensor_tensor(out=res[:], in0=tot[:], in1=rnc[:], op=OP.mult)
    nc.vector.tensor_tensor(out=res[:], in0=res[:], in1=mask[:], op=OP.mult)

    nc.sync.dma_start(loss_d[:], res[:])


def build():
    nc = bacc.Bacc(
        "TRN2", target_bir_lowering=False, debug=False, num_devices=NCORES
    )
    pred_d = nc.dram_tensor("pred", [ROWS_PAD, D], F16, kind="ExternalInput").ap()
    attr_d = nc.dram_tensor("attr", [A, ROWS_PAD], F16, kind="ExternalInput").ap()
    loss_d = nc.dram_tensor("loss", [1, 1], F32, kind="ExternalOutput").ap()
    dbg_d = nc.dram_tensor("dbg", [1, NACC], F32, kind="ExternalOutput").ap()
    with tile.TileContext(nc) as tc:
        emit_kernel(tc, pred_d, attr_d, loss_d, dbg_d)
    nc.compile()
    return nc


def shard_inputs(predictions, attr_vals, n_cores=NCORES, rows_pad=ROWS_PAD):
    rows = predictions.shape[0] // n_cores
    in_maps = []
    for c in range(n_cores):
        p = predictions[c * rows : (c + 1) * rows].astype(np.float16)
        a8 = (attr_vals[c * rows : (c + 1) * rows].astype(np.float32) * 8.0).astype(
            np.float16
        )
        pad = rows_pad - rows
        if pad:
            p = np.concatenate([p, np.zeros((pad, D), np.float16)], axis=0)
            a8 = np.concatenate(
                [a8, np.full((pad, A), 128.0, np.float16)], axis=0
            )
        in_maps.append(
            {
                "pred": np.ascontiguousarray(p),
                "attr": np.ascontiguousarray(a8.T),
            }
        )
    return in_maps


_NC_CACHE = {}


def kernel(predictions: np.ndarray, attr_vals: np.ndarray) -> np.ndarray:
    predictions = np.asarray(predictions, np.float32)
    attr_vals = np.asarray(attr_vals, np.int32)
    if "nc" not in _NC_CACHE:
        _NC_CACHE["nc"] = build()
    nc = _NC_CACHE["nc"]
    in_maps = shard_inputs(predictions, attr_vals)
    res = bass_utils.run_bass_kernel_spmd(nc, in_maps, list(range(NCORES)))
    return np.float32(res.results[0]["loss"][0, 0])
